# revision 45
# baseline (speedup 1.0000x reference)
"""AGNNConv distributed Bass kernel for 8 TRN2 NeuronCores (v11).

out = (1+eps)*feat + h,  h[d] = sum_{e: dst_e=d} p_e * norm_feat[src_e]
with p_e = edge-softmax grouped by src.

Algebra:
    w_e = exp(beta*ew_e)
    z_n = sum_{e: src_e=n} w_e            # per NODE
    g_n = feat_n / (||feat_n|| * z_n)     # per NODE
    h_d = sum_{e: dst_e=d} w_e * g[src_e]
    out = (1+eps)*feat + h

v11 (v8's ScalarE exp-in-place was the 77us pole; v9/v10 DVE splits
only added pipeline bubbles):
  64-wide one-hot: edges bucketed by (dst tile, dst half); the scatter
  matrix is [128 edges, 64 dst] so the exp area and straw bytes halve
  while matmul time is unchanged (set by the rhs free dim = D).  The
  two halves accumulate into separate [64, D] PSUM tiles at base
  partition 0; the partition shift for the upper half is done by the
  o0/out DMAs (address-based), so no tile_position tricks.
  Phase 1 (node-sharded): g64 = 64*g (bf16 -> host casts fp8) and
  o0 = (1+eps)*feat.  Host gathers g64[src_e] and scatters RAW ew_e
  into the one-hot slots with -80 fill (pure relayout); device does
  stw = exp(beta*straw - ln64) = w_e/64 one-hot-placed, then
  h = stw^T @ ge64 as PSUM-accumulating matmuls.
"""

import sys

sys.path.insert(0, "/opt/trn_rl_repo")

import numpy as np

N, E, D = 50000, 640000, 128
NCORES = 8
SH = N // NCORES            # 6250 dst nodes per core
HTILES = (SH + 127) // 128  # 49 dst tiles per core
SHP = HTILES * 128          # 6272 padded nodes per core

PAD_EW = -80.0              # exp(beta*PAD_EW) == 0 (inside ACT LUT range)
LN64 = 4.1588830833596715
GRP = 4                     # max dst tiles per DMA/ACT batch
# group plan: small first group (fast pipeline fill) and small last
# group (short tail); 4-tile groups in between
GROUPS = [[0], [1, 2, 3]] + [
    list(range(4 + 4 * k, 8 + 4 * k)) for k in range(11)
] + [[48]]
NG = len(GROUPS)            # 14
GID = {}
for _gi, _g in enumerate(GROUPS):
    for _i in _g:
        GID[_i] = _gi


def _host_prep(src, dst, edge_weight):
    """Index/layout prep only (no float math on tensor values)."""
    import ml_dtypes

    src = np.asarray(src).astype(np.int64)
    dst = np.asarray(dst).astype(np.int64)
    ew = np.asarray(edge_weight).astype(np.float32)

    # ---- per-node src-grouped edge-weight rows (for z), fixed K ----
    deg = np.bincount(src, minlength=N)
    K = int(deg.max())
    order = np.argsort(src, kind="stable")
    starts = np.zeros(N + 1, np.int64)
    np.cumsum(deg, out=starts[1:])
    slot = np.arange(E, dtype=np.int64) - starts[src[order]]
    zpad = np.full((N, K), PAD_EW, np.float32)
    zpad[src[order], slot] = ew[order]

    # per-core phase-1 z rows: [128, HTILES*K] bf16
    zrows = []
    for c in range(NCORES):
        zm = np.full((SHP, K), PAD_EW, np.float32)
        zm[:SH] = zpad[c * SH : (c + 1) * SH]
        zrows.append(
            np.ascontiguousarray(
                zm.reshape(HTILES, 128, K).transpose(1, 0, 2)
            ).reshape(128, HTILES * K).astype(ml_dtypes.bfloat16)
        )

    # ---- per-core edge bucketing by (dst tile, dst half) ----
    owner = dst // SH
    dstl = dst - owner * SH
    dtile = dstl // 128
    dbit = dstl % 128
    half = dbit // 64
    dcol = dbit % 64
    bucket = dtile * 2 + half          # [0, 2*HTILES)

    counts = np.zeros((NCORES, 2 * HTILES), np.int64)
    np.add.at(counts, (owner, bucket), 1)
    net = (counts.max(axis=0) + 127) // 128  # edge tiles per bucket
    net = np.maximum(net, 1)

    # group-local layout: buckets of group g at local cumsum offsets,
    # each group padded to GMAX edge tiles
    loff = np.zeros(2 * HTILES, np.int64)
    gsum = np.zeros(NG, np.int64)
    for g, tiles in enumerate(GROUPS):
        o = 0
        for i in tiles:
            for h in (0, 1):
                loff[2 * i + h] = o
                o += int(net[2 * i + h])
        gsum[g] = o
    GMAX = int(gsum.max())
    EPAD = NG * GMAX * 128

    core_idx = []
    for c in range(NCORES):
        m = np.nonzero(owner == c)[0]
        key = bucket[m]
        korder = np.argsort(key, kind="stable")
        me = m[korder]
        keys = key[korder]
        kb = np.r_[0, np.nonzero(np.diff(keys))[0] + 1]
        sf = np.zeros(len(keys), np.int64)
        sf[kb] = kb
        np.maximum.accumulate(sf, out=sf)
        within = np.arange(len(keys)) - sf
        gid = np.array([GID[i] for i in range(HTILES)], np.int64)
        grp = gid[keys // 2]
        pos = (grp * GMAX + loff[keys]) * 128 + within

        src_pad = np.full(EPAD, N, np.int64)      # pad edges read zero row N

        # raw-ew scatter matrix: ew_e at [e, dcol_e], -80 fill (fp8).
        # layout [NG*128, GMAX*64]: group blocks fully contiguous.
        stf = np.full((EPAD, 64), PAD_EW, np.float32)
        stf[pos, dcol[me]] = ew[me]
        straw = np.ascontiguousarray(
            stf.reshape(NG, GMAX, 128, 64).transpose(0, 2, 1, 3)
        ).reshape(NG * 128, GMAX * 64).astype(ml_dtypes.float8_e4m3)

        src_pad[pos] = src[me]
        core_idx.append((src_pad, straw))

    return zrows, core_idx, net, K, loff, gsum, GMAX


_COMPILED = {}


def _build_phase1(K):
    import concourse.bass as bass
    import concourse.bacc as bacc
    from concourse import mybir, tile

    f32 = mybir.dt.float32
    bf16 = mybir.dt.bfloat16
    i32 = mybir.dt.int32
    AF = mybir.ActivationFunctionType
    ALU = mybir.AluOpType
    X = mybir.AxisListType.X

    NT = HTILES       # 49 node tiles
    CH = 17           # node tiles per pipeline chunk (3 chunks: 17+16+16)
    NCH = (NT + CH - 1) // CH

    nc = bacc.Bacc(None, debug=False)
    fm_ext = nc.dram_tensor("feat_my", [128, NT * D], bf16, kind="ExternalInput")
    zr_ext = nc.dram_tensor("zrow", [128, NT * K], bf16, kind="ExternalInput")
    beta_ext = nc.dram_tensor("beta128", [128, 1], f32, kind="ExternalInput")
    eps_ext = nc.dram_tensor("eps128", [128, 1], f32, kind="ExternalInput")
    g_ext = nc.dram_tensor("g64", [128, NT * D], bf16, kind="ExternalOutput")
    o0_ext = nc.dram_tensor("o0", [128, NT * D], bf16, kind="ExternalOutput")

    with tile.TileContext(nc) as tc:
        with (
            tc.tile_pool(name="pp", bufs=1) as pp,
            tc.tile_pool(name="fmp", bufs=2) as fmp,
            tc.tile_pool(name="zrp", bufs=2) as zrp,
            tc.tile_pool(name="sqp", bufs=2) as sqp,
            tc.tile_pool(name="smp", bufs=2 * 8) as smp,
            tc.tile_pool(name="gp", bufs=2) as gp,
            tc.tile_pool(name="op", bufs=2) as op,
        ):
            beta_b = pp.tile([128, 1], f32, tag="beta_b")
            nc.sync.dma_start(out=beta_b[:], in_=beta_ext[:])
            ep1_b = pp.tile([128, 1], f32, tag="ep1_b")
            nc.sync.dma_start(out=ep1_b[:], in_=eps_ext[:])
            nc.vector.tensor_scalar_add(ep1_b[:], ep1_b[:], 1.0)

            for ci in range(NCH):
                t0 = ci * CH
                nt = min(CH, NT - t0)
                fm = fmp.tile([128, CH, D], bf16, tag="fm")
                nc.sync.dma_start(
                    out=fm[:, :nt, :].rearrange("p a b -> p (a b)"),
                    in_=fm_ext[:, t0 * D : (t0 + nt) * D],
                )
                # zrow on the scalar HWDGE ring: feeds the first ACT
                # without queueing behind fm
                zr = zrp.tile([128, CH, K], bf16, tag="zr")
                nc.scalar.dma_start(
                    out=zr[:, :nt, :].rearrange("p a b -> p (a b)"),
                    in_=zr_ext[:, t0 * K : (t0 + nt) * K],
                )

                # o0 = (1+eps)*feat (ACT, independent of the norm chain --
                # emitted first so it overlaps and doesn't extend the tail)
                o0 = op.tile([128, CH, D], bf16, tag="o0")
                nc.scalar.activation(
                    o0[:, :nt, :].rearrange("p a b -> p (a b)"),
                    fm[:, :nt, :].rearrange("p a b -> p (a b)"),
                    AF.Copy,
                    scale=ep1_b[:],
                )
                nc.sync.dma_start(
                    out=o0_ext[:, t0 * D : (t0 + nt) * D],
                    in_=o0[:, :nt, :].rearrange("p a b -> p (a b)"),
                )

                # z = sum_k exp(beta * zrow_k)
                zx = zrp.tile([128, CH, K], bf16, tag="zx")
                nc.scalar.activation(
                    zx[:, :nt, :].rearrange("p a b -> p (a b)"),
                    zr[:, :nt, :].rearrange("p a b -> p (a b)"),
                    AF.Exp,
                    scale=beta_b[:],
                )
                z = smp.tile([128, CH], f32, tag="z")
                nc.vector.tensor_reduce(z[:, :nt], zx[:, :nt, :], X, ALU.add)

                # ss = ||feat||^2 per node (bf16 squares -> 2x mode)
                sq = sqp.tile([128, CH, D], bf16, tag="sq")
                nc.vector.tensor_tensor(
                    sq[:, :nt, :].rearrange("p a b -> p (a b)"),
                    fm[:, :nt, :].rearrange("p a b -> p (a b)"),
                    fm[:, :nt, :].rearrange("p a b -> p (a b)"),
                    ALU.mult,
                )
                ss = smp.tile([128, CH], f32, tag="ss")
                nc.vector.tensor_reduce(ss[:, :nt], sq[:, :nt, :], X, ALU.add)

                # rz = 64/sqrt(ss*z^2): one bit-hack rsqrt + Newton on v
                v = smp.tile([128, CH], f32, tag="v")
                nc.vector.tensor_tensor(v[:, :nt], ss[:, :nt], z[:, :nt], ALU.mult)
                nc.vector.tensor_tensor(v[:, :nt], v[:, :nt], z[:, :nt], ALU.mult)
                y0 = smp.tile([128, CH], f32, tag="y0")
                nc.vector.tensor_scalar(
                    y0[:, :nt].bitcast(i32), v[:, :nt].bitcast(i32),
                    1, -1, op0=ALU.arith_shift_right, op1=ALU.bitwise_xor,
                )
                nc.vector.tensor_scalar(
                    y0[:, :nt].bitcast(i32), y0[:, :nt].bitcast(i32),
                    0x5F3759E0, None, op0=ALU.add,
                )
                u = smp.tile([128, CH], f32, tag="u")
                nc.vector.tensor_tensor(u[:, :nt], y0[:, :nt], y0[:, :nt], ALU.mult)
                nc.vector.tensor_tensor(u[:, :nt], u[:, :nt], v[:, :nt], ALU.mult)
                # fold the 64x: y = y*(96 - 32*u) = 64*y*(1.5 - 0.5*u)
                nc.vector.tensor_scalar(
                    u[:, :nt], u[:, :nt], -32.0, 96.0, op0=ALU.mult, op1=ALU.add
                )
                rz = smp.tile([128, CH], f32, tag="rz")
                nc.vector.tensor_tensor(rz[:, :nt], y0[:, :nt], u[:, :nt], ALU.mult)

                # g64 = feat * rz
                g = gp.tile([128, CH, D], bf16, tag="g")
                rzb = rz[:, :nt].unsqueeze(2).broadcast_to([128, nt, D])
                nc.vector.tensor_tensor(g[:, :nt, :], fm[:, :nt, :], rzb, ALU.mult)
                nc.sync.dma_start(
                    out=g_ext[:, t0 * D : (t0 + nt) * D],
                    in_=g[:, :nt, :].rearrange("p a b -> p (a b)"),
                )

    nc.finalize()
    return nc


def _build_phase2(net, loff, gsum, GMAX):
    import concourse.bass as bass
    import concourse.bacc as bacc
    from concourse import mybir, tile

    f32 = mybir.dt.float32
    bf16 = mybir.dt.bfloat16
    f8 = mybir.dt.float8e4
    AF = mybir.ActivationFunctionType
    ALU = mybir.AluOpType

    NT = HTILES

    nc = bacc.Bacc(None, debug=False)
    ge_ext = nc.dram_tensor("ge", [NG * 128, GMAX * D], f8, kind="ExternalInput")
    st_ext = nc.dram_tensor("straw", [NG * 128, GMAX * 64], f8, kind="ExternalInput")
    o0_ext = nc.dram_tensor("o0", [128, NT * D], bf16, kind="ExternalInput")
    beta_ext = nc.dram_tensor("beta128", [128, 1], f32, kind="ExternalInput")
    # out layout [64, (group, half, grp-local, D)]: one DMA per group
    out_ext = nc.dram_tensor(
        "out", [64, NG * 2 * GRP * D], bf16, kind="ExternalOutput"
    )

    with tile.TileContext(nc) as tc:
        with (
            tc.tile_pool(name="persist", bufs=1) as pp,
            tc.tile_pool(name="gep", bufs=4) as gepool,
            tc.tile_pool(name="stp", bufs=4) as stpool,
            tc.tile_pool(name="stw", bufs=5) as stwpool,
            tc.tile_pool(name="outp", bufs=3) as opool,
            tc.tile_pool(name="hpsum", bufs=8, space="PSUM") as hpsum,
        ):
            beta_b = pp.tile([128, 1], f32, tag="beta_b")
            nc.sync.dma_start(out=beta_b[:], in_=beta_ext[:])
            bl64 = pp.tile([128, 1], f32, tag="bl64")
            nc.vector.memset(bl64[:], -LN64)

            # (1+eps)*feat resident in thirds, loads staggered across early
            # groups so the o0 traffic never saturates the SDMA engines at
            # startup (a consolidated 2.4MB burst stalled group 1 by 14us).
            # A-half reads rows 0:64 of the full-partition part tiles; the
            # B-half needs rows 64:128 shifted to partitions 0:64, done by
            # the HBM load itself (DMA is address-based).
            O0SPLIT = [(0, 49, 0, 0)]
            o0a_parts = []
            o0b_parts = []
            for ps, pe, _, _ in O0SPLIT:
                o0a_p = pp.tile([128, pe - ps, D], bf16, tag=f"o0a{ps}")
                o0b_p = pp.tile([64, pe - ps, D], bf16, tag=f"o0b{ps}")
                o0a_parts.append(o0a_p)
                o0b_parts.append(o0b_p)

            def o0_lookup(i):
                for pi, (ps, pe, _, _) in enumerate(O0SPLIT):
                    if i < pe:
                        return pi, i - ps
                raise AssertionError

            for g, tiles in enumerate(GROUPS):
                i0 = tiles[0]
                nw = len(tiles)
                gnh = int(gsum[g])

                # straw rides the scalar-issued HWDGE ring so it never
                # queues behind the bigger ge stream -> ACT stays ahead
                straw = stpool.tile([128, GMAX, 64], f8, tag="straw")
                nc.scalar.dma_start(
                    out=straw[:, :gnh, :].rearrange("p a b -> p (a b)"),
                    in_=st_ext[g * 128 : (g + 1) * 128, : gnh * 64],
                )
                ge = gepool.tile([128, GMAX, D], f8, tag="ge")
                nc.sync.dma_start(
                    out=ge[:, :gnh, :].rearrange("p a b -> p (a b)"),
                    in_=ge_ext[g * 128 : (g + 1) * 128, : gnh * D],
                )
                for pi, (ps, pe, ga, gb) in enumerate(O0SPLIT):
                    if g == ga:
                        nc.scalar.dma_start(
                            out=o0a_parts[pi][:].rearrange("p a b -> p (a b)"),
                            in_=o0_ext[:, ps * D : pe * D],
                        )
                    if g == gb:
                        nc.sync.dma_start(
                            out=o0b_parts[pi][:].rearrange("p a b -> p (a b)"),
                            in_=o0_ext[64:128, ps * D : pe * D],
                        )

                # stw = exp(beta*straw - ln64): w_e/64 one-hot-placed
                stw = stwpool.tile([128, GMAX, 64], bf16, tag="stw")
                nc.scalar.activation(
                    stw[:, :gnh, :].rearrange("p a b -> p (a b)"),
                    straw[:, :gnh, :].rearrange("p a b -> p (a b)"),
                    AF.Exp,
                    bias=bl64[:],
                    scale=beta_b[:],
                )

                og = opool.tile([64, 2, GRP, D], bf16, tag="og")
                for li, i in enumerate(tiles):
                    for h in (0, 1):
                        nh = int(net[2 * i + h])
                        lo = int(loff[2 * i + h])
                        hp = hpsum.tile([64, D], f32, tag="hp")
                        for t in range(nh):
                            nc.tensor.matmul(
                                hp[:],
                                stw[:, lo + t, :],
                                ge[:, lo + t, :],
                                start=(t == 0),
                                stop=(t == nh - 1),
                            )
                        pi, io = o0_lookup(i)
                        o0src = (
                            o0a_parts[pi][0:64, io, :]
                            if h == 0
                            else o0b_parts[pi][:, io, :]
                        )
                        nc.vector.tensor_tensor(
                            og[:, h, li, :], o0src, hp[:], ALU.add
                        )
                nc.sync.dma_start(
                    out=out_ext[:, g * 2 * GRP * D : (g + 1) * 2 * GRP * D],
                    in_=og[:].rearrange("p a b c -> p (a b c)"),
                )

    nc.finalize()
    return nc


def kernel(feat, edge_weight, src, dst, beta, eps):
    from concourse.bass_utils import run_bass_kernel_spmd
    import ml_dtypes

    feat = np.asarray(feat, dtype=np.float32)
    ew = np.asarray(edge_weight, dtype=np.float32)
    beta = np.asarray(beta, dtype=np.float32)
    eps = np.asarray(eps, dtype=np.float32)

    zrows, core_idx, net, K, loff, gsum, GMAX = _host_prep(src, dst, ew)

    key = (K, GMAX, tuple(int(x) for x in net))
    if key not in _COMPILED:
        _COMPILED[key] = (
            _build_phase1(K),
            _build_phase2(net, loff, gsum, GMAX),
        )
    nc1, nc2 = _COMPILED[key]

    beta128 = np.ascontiguousarray(np.broadcast_to(beta.reshape(1, 1), (128, 1)))
    eps128 = np.ascontiguousarray(np.broadcast_to(eps.reshape(1, 1), (128, 1)))

    # ---------------- phase 1: per-node g64, o0 ----------------
    in1 = []
    for c in range(NCORES):
        fmp = np.zeros((SHP, D), np.float32)
        fmp[:SH] = feat[c * SH : (c + 1) * SH]
        fmt = np.ascontiguousarray(
            fmp.reshape(HTILES, 128, D).transpose(1, 0, 2)
        ).reshape(128, HTILES * D).astype(ml_dtypes.bfloat16)
        in1.append(
            {"feat_my": fmt, "zrow": zrows[c], "beta128": beta128,
             "eps128": eps128}
        )

    res1 = run_bass_kernel_spmd(nc1, in1, core_ids=list(range(NCORES)))
    gfull = np.empty((N + 1, D), dtype=ml_dtypes.float8_e4m3)
    o0s = []
    for c in range(NCORES):
        gc = np.asarray(res1.results[c]["g64"]).reshape(128, HTILES, D)
        gfull[c * SH : (c + 1) * SH] = (
            gc.transpose(1, 0, 2).reshape(SHP, D)[:SH].astype(ml_dtypes.float8_e4m3)
        )
        o0s.append(np.asarray(res1.results[c]["o0"]))
    gfull[N] = 0  # pad row

    # ---------------- host gather of g64[src_e] ----------------
    in2 = []
    for c in range(NCORES):
        src_pad, straw = core_idx[c]
        ge = np.ascontiguousarray(
            gfull[src_pad].reshape(NG, GMAX, 128, D).transpose(0, 2, 1, 3)
        ).reshape(NG * 128, GMAX * D)
        in2.append(
            {"ge": ge, "straw": straw, "o0": o0s[c], "beta128": beta128}
        )

    res2 = run_bass_kernel_spmd(nc2, in2, core_ids=list(range(NCORES)))
    out = np.empty((N, D), np.float32)
    full = np.empty((SHP, D), np.float32)
    for c in range(NCORES):
        # [64, NG, 2, GRP, D] -> node i*128 + h*64 + p
        oc = np.asarray(res2.results[c]["out"]).reshape(64, NG, 2, GRP, D)
        for gi, tiles in enumerate(GROUPS):
            for li, i in enumerate(tiles):
                for h in (0, 1):
                    full[i * 128 + h * 64 : i * 128 + (h + 1) * 64] = oc[
                        :, gi, h, li, :
                    ]
        out[c * SH : (c + 1) * SH] = full[:SH]
    return out


# revision 46
# speedup vs baseline: 1.0326x; 1.0326x over previous
"""AGNNConv distributed Bass kernel for 8 TRN2 NeuronCores (v11).

out = (1+eps)*feat + h,  h[d] = sum_{e: dst_e=d} p_e * norm_feat[src_e]
with p_e = edge-softmax grouped by src.

Algebra:
    w_e = exp(beta*ew_e)
    z_n = sum_{e: src_e=n} w_e            # per NODE
    g_n = feat_n / (||feat_n|| * z_n)     # per NODE
    h_d = sum_{e: dst_e=d} w_e * g[src_e]
    out = (1+eps)*feat + h

v11 (v8's ScalarE exp-in-place was the 77us pole; v9/v10 DVE splits
only added pipeline bubbles):
  64-wide one-hot: edges bucketed by (dst tile, dst half); the scatter
  matrix is [128 edges, 64 dst] so the exp area and straw bytes halve
  while matmul time is unchanged (set by the rhs free dim = D).  The
  two halves accumulate into separate [64, D] PSUM tiles at base
  partition 0; the partition shift for the upper half is done by the
  o0/out DMAs (address-based), so no tile_position tricks.
  Phase 1 (node-sharded): g64 = 64*g (bf16 -> host casts fp8) and
  o0 = (1+eps)*feat.  Host gathers g64[src_e] and scatters RAW ew_e
  into the one-hot slots with -80 fill (pure relayout); device does
  stw = exp(beta*straw - ln64) = w_e/64 one-hot-placed, then
  h = stw^T @ ge64 as PSUM-accumulating matmuls.
"""

import sys

sys.path.insert(0, "/opt/trn_rl_repo")

import numpy as np

N, E, D = 50000, 640000, 128
NCORES = 8
SH = N // NCORES            # 6250 dst nodes per core
HTILES = (SH + 127) // 128  # 49 dst tiles per core
SHP = HTILES * 128          # 6272 padded nodes per core

PAD_EW = -80.0              # exp(beta*PAD_EW) == 0 (inside ACT LUT range)
LN64 = 4.1588830833596715
GRP = 8                     # max dst tiles per DMA/ACT batch
# group plan: small first groups (fast pipeline fill), then 8-tile
# groups whose ~1.9MB ge chunks run near peak DMA rate
GROUPS = [[0], [1, 2, 3]] + [
    list(range(4 + 8 * k, 12 + 8 * k)) for k in range(5)
] + [[44, 45, 46, 47, 48]]
NG = len(GROUPS)            # 8
GID = {}
for _gi, _g in enumerate(GROUPS):
    for _i in _g:
        GID[_i] = _gi


def _host_prep(src, dst, edge_weight):
    """Index/layout prep only (no float math on tensor values)."""
    import ml_dtypes

    src = np.asarray(src).astype(np.int64)
    dst = np.asarray(dst).astype(np.int64)
    ew = np.asarray(edge_weight).astype(np.float32)

    # ---- per-node src-grouped edge-weight rows (for z), fixed K ----
    deg = np.bincount(src, minlength=N)
    K = int(deg.max())
    order = np.argsort(src, kind="stable")
    starts = np.zeros(N + 1, np.int64)
    np.cumsum(deg, out=starts[1:])
    slot = np.arange(E, dtype=np.int64) - starts[src[order]]
    zpad = np.full((N, K), PAD_EW, np.float32)
    zpad[src[order], slot] = ew[order]

    # per-core phase-1 z rows: [128, HTILES*K] bf16
    zrows = []
    for c in range(NCORES):
        zm = np.full((SHP, K), PAD_EW, np.float32)
        zm[:SH] = zpad[c * SH : (c + 1) * SH]
        zrows.append(
            np.ascontiguousarray(
                zm.reshape(HTILES, 128, K).transpose(1, 0, 2)
            ).reshape(128, HTILES * K).astype(ml_dtypes.bfloat16)
        )

    # ---- per-core edge bucketing by (dst tile, dst half) ----
    owner = dst // SH
    dstl = dst - owner * SH
    dtile = dstl // 128
    dbit = dstl % 128
    half = dbit // 64
    dcol = dbit % 64
    bucket = dtile * 2 + half          # [0, 2*HTILES)

    counts = np.zeros((NCORES, 2 * HTILES), np.int64)
    np.add.at(counts, (owner, bucket), 1)
    net = (counts.max(axis=0) + 127) // 128  # edge tiles per bucket
    net = np.maximum(net, 1)

    # group-local layout: buckets of group g at local cumsum offsets,
    # each group padded to GMAX edge tiles
    loff = np.zeros(2 * HTILES, np.int64)
    gsum = np.zeros(NG, np.int64)
    for g, tiles in enumerate(GROUPS):
        o = 0
        for i in tiles:
            for h in (0, 1):
                loff[2 * i + h] = o
                o += int(net[2 * i + h])
        gsum[g] = o
    GMAX = int(gsum.max())
    EPAD = NG * GMAX * 128

    core_idx = []
    for c in range(NCORES):
        m = np.nonzero(owner == c)[0]
        key = bucket[m]
        korder = np.argsort(key, kind="stable")
        me = m[korder]
        keys = key[korder]
        kb = np.r_[0, np.nonzero(np.diff(keys))[0] + 1]
        sf = np.zeros(len(keys), np.int64)
        sf[kb] = kb
        np.maximum.accumulate(sf, out=sf)
        within = np.arange(len(keys)) - sf
        gid = np.array([GID[i] for i in range(HTILES)], np.int64)
        grp = gid[keys // 2]
        pos = (grp * GMAX + loff[keys]) * 128 + within

        src_pad = np.full(EPAD, N, np.int64)      # pad edges read zero row N

        # raw-ew scatter matrix: ew_e at [e, dcol_e], -80 fill (fp8).
        # layout [NG*128, GMAX*64]: group blocks fully contiguous.
        stf = np.full((EPAD, 64), PAD_EW, np.float32)
        stf[pos, dcol[me]] = ew[me]
        straw = np.ascontiguousarray(
            stf.reshape(NG, GMAX, 128, 64).transpose(0, 2, 1, 3)
        ).reshape(NG * 128, GMAX * 64).astype(ml_dtypes.float8_e4m3)

        src_pad[pos] = src[me]
        core_idx.append((src_pad, straw))

    return zrows, core_idx, net, K, loff, gsum, GMAX


_COMPILED = {}


def _build_phase1(K):
    import concourse.bass as bass
    import concourse.bacc as bacc
    from concourse import mybir, tile

    f32 = mybir.dt.float32
    bf16 = mybir.dt.bfloat16
    i32 = mybir.dt.int32
    AF = mybir.ActivationFunctionType
    ALU = mybir.AluOpType
    X = mybir.AxisListType.X

    NT = HTILES       # 49 node tiles
    CH = 17           # node tiles per pipeline chunk (3 chunks: 17+16+16)
    NCH = (NT + CH - 1) // CH

    nc = bacc.Bacc(None, debug=False)
    fm_ext = nc.dram_tensor("feat_my", [128, NT * D], bf16, kind="ExternalInput")
    zr_ext = nc.dram_tensor("zrow", [128, NT * K], bf16, kind="ExternalInput")
    beta_ext = nc.dram_tensor("beta128", [128, 1], f32, kind="ExternalInput")
    eps_ext = nc.dram_tensor("eps128", [128, 1], f32, kind="ExternalInput")
    g_ext = nc.dram_tensor("g64", [128, NT * D], bf16, kind="ExternalOutput")
    o0_ext = nc.dram_tensor("o0", [128, NT * D], bf16, kind="ExternalOutput")

    with tile.TileContext(nc) as tc:
        with (
            tc.tile_pool(name="pp", bufs=1) as pp,
            tc.tile_pool(name="fmp", bufs=2) as fmp,
            tc.tile_pool(name="zrp", bufs=2) as zrp,
            tc.tile_pool(name="sqp", bufs=2) as sqp,
            tc.tile_pool(name="smp", bufs=2 * 8) as smp,
            tc.tile_pool(name="gp", bufs=2) as gp,
            tc.tile_pool(name="op", bufs=2) as op,
        ):
            beta_b = pp.tile([128, 1], f32, tag="beta_b")
            nc.sync.dma_start(out=beta_b[:], in_=beta_ext[:])
            ep1_b = pp.tile([128, 1], f32, tag="ep1_b")
            nc.sync.dma_start(out=ep1_b[:], in_=eps_ext[:])
            nc.vector.tensor_scalar_add(ep1_b[:], ep1_b[:], 1.0)

            for ci in range(NCH):
                t0 = ci * CH
                nt = min(CH, NT - t0)
                fm = fmp.tile([128, CH, D], bf16, tag="fm")
                nc.sync.dma_start(
                    out=fm[:, :nt, :].rearrange("p a b -> p (a b)"),
                    in_=fm_ext[:, t0 * D : (t0 + nt) * D],
                )
                # zrow on the scalar HWDGE ring: feeds the first ACT
                # without queueing behind fm
                zr = zrp.tile([128, CH, K], bf16, tag="zr")
                nc.scalar.dma_start(
                    out=zr[:, :nt, :].rearrange("p a b -> p (a b)"),
                    in_=zr_ext[:, t0 * K : (t0 + nt) * K],
                )

                # o0 = (1+eps)*feat (ACT, independent of the norm chain --
                # emitted first so it overlaps and doesn't extend the tail)
                o0 = op.tile([128, CH, D], bf16, tag="o0")
                nc.scalar.activation(
                    o0[:, :nt, :].rearrange("p a b -> p (a b)"),
                    fm[:, :nt, :].rearrange("p a b -> p (a b)"),
                    AF.Copy,
                    scale=ep1_b[:],
                )
                nc.sync.dma_start(
                    out=o0_ext[:, t0 * D : (t0 + nt) * D],
                    in_=o0[:, :nt, :].rearrange("p a b -> p (a b)"),
                )

                # z = sum_k exp(beta * zrow_k)
                zx = zrp.tile([128, CH, K], bf16, tag="zx")
                nc.scalar.activation(
                    zx[:, :nt, :].rearrange("p a b -> p (a b)"),
                    zr[:, :nt, :].rearrange("p a b -> p (a b)"),
                    AF.Exp,
                    scale=beta_b[:],
                )
                z = smp.tile([128, CH], f32, tag="z")
                nc.vector.tensor_reduce(z[:, :nt], zx[:, :nt, :], X, ALU.add)

                # ss = ||feat||^2 per node (bf16 squares -> 2x mode)
                sq = sqp.tile([128, CH, D], bf16, tag="sq")
                nc.vector.tensor_tensor(
                    sq[:, :nt, :].rearrange("p a b -> p (a b)"),
                    fm[:, :nt, :].rearrange("p a b -> p (a b)"),
                    fm[:, :nt, :].rearrange("p a b -> p (a b)"),
                    ALU.mult,
                )
                ss = smp.tile([128, CH], f32, tag="ss")
                nc.vector.tensor_reduce(ss[:, :nt], sq[:, :nt, :], X, ALU.add)

                # rz = 64/sqrt(ss*z^2): one bit-hack rsqrt + Newton on v
                v = smp.tile([128, CH], f32, tag="v")
                nc.vector.tensor_tensor(v[:, :nt], ss[:, :nt], z[:, :nt], ALU.mult)
                nc.vector.tensor_tensor(v[:, :nt], v[:, :nt], z[:, :nt], ALU.mult)
                y0 = smp.tile([128, CH], f32, tag="y0")
                nc.vector.tensor_scalar(
                    y0[:, :nt].bitcast(i32), v[:, :nt].bitcast(i32),
                    1, -1, op0=ALU.arith_shift_right, op1=ALU.bitwise_xor,
                )
                nc.vector.tensor_scalar(
                    y0[:, :nt].bitcast(i32), y0[:, :nt].bitcast(i32),
                    0x5F3759E0, None, op0=ALU.add,
                )
                u = smp.tile([128, CH], f32, tag="u")
                nc.vector.tensor_tensor(u[:, :nt], y0[:, :nt], y0[:, :nt], ALU.mult)
                nc.vector.tensor_tensor(u[:, :nt], u[:, :nt], v[:, :nt], ALU.mult)
                # fold the 64x: y = y*(96 - 32*u) = 64*y*(1.5 - 0.5*u)
                nc.vector.tensor_scalar(
                    u[:, :nt], u[:, :nt], -32.0, 96.0, op0=ALU.mult, op1=ALU.add
                )
                rz = smp.tile([128, CH], f32, tag="rz")
                nc.vector.tensor_tensor(rz[:, :nt], y0[:, :nt], u[:, :nt], ALU.mult)

                # g64 = feat * rz
                g = gp.tile([128, CH, D], bf16, tag="g")
                rzb = rz[:, :nt].unsqueeze(2).broadcast_to([128, nt, D])
                nc.vector.tensor_tensor(g[:, :nt, :], fm[:, :nt, :], rzb, ALU.mult)
                nc.sync.dma_start(
                    out=g_ext[:, t0 * D : (t0 + nt) * D],
                    in_=g[:, :nt, :].rearrange("p a b -> p (a b)"),
                )

    nc.finalize()
    return nc


def _build_phase2(net, loff, gsum, GMAX):
    import concourse.bass as bass
    import concourse.bacc as bacc
    from concourse import mybir, tile

    f32 = mybir.dt.float32
    bf16 = mybir.dt.bfloat16
    f8 = mybir.dt.float8e4
    AF = mybir.ActivationFunctionType
    ALU = mybir.AluOpType

    NT = HTILES

    nc = bacc.Bacc(None, debug=False)
    ge_ext = nc.dram_tensor("ge", [NG * 128, GMAX * D], f8, kind="ExternalInput")
    st_ext = nc.dram_tensor("straw", [NG * 128, GMAX * 64], f8, kind="ExternalInput")
    o0_ext = nc.dram_tensor("o0", [128, NT * D], bf16, kind="ExternalInput")
    beta_ext = nc.dram_tensor("beta128", [128, 1], f32, kind="ExternalInput")
    # out layout [64, (group, half, grp-local, D)]: one DMA per group
    out_ext = nc.dram_tensor(
        "out", [64, NG * 2 * GRP * D], bf16, kind="ExternalOutput"
    )

    with tile.TileContext(nc) as tc:
        with (
            tc.tile_pool(name="persist", bufs=1) as pp,
            tc.tile_pool(name="gep", bufs=4) as gepool,
            tc.tile_pool(name="stp", bufs=4) as stpool,
            tc.tile_pool(name="stw", bufs=5) as stwpool,
            tc.tile_pool(name="outp", bufs=3) as opool,
            tc.tile_pool(name="hpsum", bufs=8, space="PSUM") as hpsum,
        ):
            beta_b = pp.tile([128, 1], f32, tag="beta_b")
            nc.sync.dma_start(out=beta_b[:], in_=beta_ext[:])
            bl64 = pp.tile([128, 1], f32, tag="bl64")
            nc.vector.memset(bl64[:], -LN64)

            # (1+eps)*feat resident in thirds, loads staggered across early
            # groups so the o0 traffic never saturates the SDMA engines at
            # startup (a consolidated 2.4MB burst stalled group 1 by 14us).
            # A-half reads rows 0:64 of the full-partition part tiles; the
            # B-half needs rows 64:128 shifted to partitions 0:64, done by
            # the HBM load itself (DMA is address-based).
            O0SPLIT = [(0, 49, 0, 0)]
            o0a_parts = []
            o0b_parts = []
            for ps, pe, _, _ in O0SPLIT:
                o0a_p = pp.tile([128, pe - ps, D], bf16, tag=f"o0a{ps}")
                o0b_p = pp.tile([64, pe - ps, D], bf16, tag=f"o0b{ps}")
                o0a_parts.append(o0a_p)
                o0b_parts.append(o0b_p)

            def o0_lookup(i):
                for pi, (ps, pe, _, _) in enumerate(O0SPLIT):
                    if i < pe:
                        return pi, i - ps
                raise AssertionError

            for g, tiles in enumerate(GROUPS):
                i0 = tiles[0]
                nw = len(tiles)
                gnh = int(gsum[g])

                # straw rides the scalar-issued HWDGE ring so it never
                # queues behind the bigger ge stream -> ACT stays ahead
                straw = stpool.tile([128, GMAX, 64], f8, tag="straw")
                nc.scalar.dma_start(
                    out=straw[:, :gnh, :].rearrange("p a b -> p (a b)"),
                    in_=st_ext[g * 128 : (g + 1) * 128, : gnh * 64],
                )
                ge = gepool.tile([128, GMAX, D], f8, tag="ge")
                nc.sync.dma_start(
                    out=ge[:, :gnh, :].rearrange("p a b -> p (a b)"),
                    in_=ge_ext[g * 128 : (g + 1) * 128, : gnh * D],
                )
                for pi, (ps, pe, ga, gb) in enumerate(O0SPLIT):
                    if g == ga:
                        nc.scalar.dma_start(
                            out=o0a_parts[pi][:].rearrange("p a b -> p (a b)"),
                            in_=o0_ext[:, ps * D : pe * D],
                        )
                    if g == gb:
                        nc.sync.dma_start(
                            out=o0b_parts[pi][:].rearrange("p a b -> p (a b)"),
                            in_=o0_ext[64:128, ps * D : pe * D],
                        )

                # stw = exp(beta*straw - ln64): w_e/64 one-hot-placed
                stw = stwpool.tile([128, GMAX, 64], bf16, tag="stw")
                nc.scalar.activation(
                    stw[:, :gnh, :].rearrange("p a b -> p (a b)"),
                    straw[:, :gnh, :].rearrange("p a b -> p (a b)"),
                    AF.Exp,
                    bias=bl64[:],
                    scale=beta_b[:],
                )

                og = opool.tile([64, 2, GRP, D], bf16, tag="og")
                for li, i in enumerate(tiles):
                    for h in (0, 1):
                        nh = int(net[2 * i + h])
                        lo = int(loff[2 * i + h])
                        hp = hpsum.tile([64, D], f32, tag="hp")
                        for t in range(nh):
                            nc.tensor.matmul(
                                hp[:],
                                stw[:, lo + t, :],
                                ge[:, lo + t, :],
                                start=(t == 0),
                                stop=(t == nh - 1),
                            )
                        pi, io = o0_lookup(i)
                        o0src = (
                            o0a_parts[pi][0:64, io, :]
                            if h == 0
                            else o0b_parts[pi][:, io, :]
                        )
                        nc.vector.tensor_tensor(
                            og[:, h, li, :], o0src, hp[:], ALU.add
                        )
                nc.sync.dma_start(
                    out=out_ext[:, g * 2 * GRP * D : (g + 1) * 2 * GRP * D],
                    in_=og[:].rearrange("p a b c -> p (a b c)"),
                )

    nc.finalize()
    return nc


def kernel(feat, edge_weight, src, dst, beta, eps):
    from concourse.bass_utils import run_bass_kernel_spmd
    import ml_dtypes

    feat = np.asarray(feat, dtype=np.float32)
    ew = np.asarray(edge_weight, dtype=np.float32)
    beta = np.asarray(beta, dtype=np.float32)
    eps = np.asarray(eps, dtype=np.float32)

    zrows, core_idx, net, K, loff, gsum, GMAX = _host_prep(src, dst, ew)

    key = (K, GMAX, tuple(int(x) for x in net))
    if key not in _COMPILED:
        _COMPILED[key] = (
            _build_phase1(K),
            _build_phase2(net, loff, gsum, GMAX),
        )
    nc1, nc2 = _COMPILED[key]

    beta128 = np.ascontiguousarray(np.broadcast_to(beta.reshape(1, 1), (128, 1)))
    eps128 = np.ascontiguousarray(np.broadcast_to(eps.reshape(1, 1), (128, 1)))

    # ---------------- phase 1: per-node g64, o0 ----------------
    in1 = []
    for c in range(NCORES):
        fmp = np.zeros((SHP, D), np.float32)
        fmp[:SH] = feat[c * SH : (c + 1) * SH]
        fmt = np.ascontiguousarray(
            fmp.reshape(HTILES, 128, D).transpose(1, 0, 2)
        ).reshape(128, HTILES * D).astype(ml_dtypes.bfloat16)
        in1.append(
            {"feat_my": fmt, "zrow": zrows[c], "beta128": beta128,
             "eps128": eps128}
        )

    res1 = run_bass_kernel_spmd(nc1, in1, core_ids=list(range(NCORES)))
    gfull = np.empty((N + 1, D), dtype=ml_dtypes.float8_e4m3)
    o0s = []
    for c in range(NCORES):
        gc = np.asarray(res1.results[c]["g64"]).reshape(128, HTILES, D)
        gfull[c * SH : (c + 1) * SH] = (
            gc.transpose(1, 0, 2).reshape(SHP, D)[:SH].astype(ml_dtypes.float8_e4m3)
        )
        o0s.append(np.asarray(res1.results[c]["o0"]))
    gfull[N] = 0  # pad row

    # ---------------- host gather of g64[src_e] ----------------
    in2 = []
    for c in range(NCORES):
        src_pad, straw = core_idx[c]
        ge = np.ascontiguousarray(
            gfull[src_pad].reshape(NG, GMAX, 128, D).transpose(0, 2, 1, 3)
        ).reshape(NG * 128, GMAX * D)
        in2.append(
            {"ge": ge, "straw": straw, "o0": o0s[c], "beta128": beta128}
        )

    res2 = run_bass_kernel_spmd(nc2, in2, core_ids=list(range(NCORES)))
    out = np.empty((N, D), np.float32)
    full = np.empty((SHP, D), np.float32)
    for c in range(NCORES):
        # [64, NG, 2, GRP, D] -> node i*128 + h*64 + p
        oc = np.asarray(res2.results[c]["out"]).reshape(64, NG, 2, GRP, D)
        for gi, tiles in enumerate(GROUPS):
            for li, i in enumerate(tiles):
                for h in (0, 1):
                    full[i * 128 + h * 64 : i * 128 + (h + 1) * 64] = oc[
                        :, gi, h, li, :
                    ]
        out[c * SH : (c + 1) * SH] = full[:SH]
    return out


# revision 48
# speedup vs baseline: 1.0456x; 1.0125x over previous
"""AGNNConv distributed Bass kernel for 8 TRN2 NeuronCores (v11).

out = (1+eps)*feat + h,  h[d] = sum_{e: dst_e=d} p_e * norm_feat[src_e]
with p_e = edge-softmax grouped by src.

Algebra:
    w_e = exp(beta*ew_e)
    z_n = sum_{e: src_e=n} w_e            # per NODE
    g_n = feat_n / (||feat_n|| * z_n)     # per NODE
    h_d = sum_{e: dst_e=d} w_e * g[src_e]
    out = (1+eps)*feat + h

v11 (v8's ScalarE exp-in-place was the 77us pole; v9/v10 DVE splits
only added pipeline bubbles):
  64-wide one-hot: edges bucketed by (dst tile, dst half); the scatter
  matrix is [128 edges, 64 dst] so the exp area and straw bytes halve
  while matmul time is unchanged (set by the rhs free dim = D).  The
  two halves accumulate into separate [64, D] PSUM tiles at base
  partition 0; the partition shift for the upper half is done by the
  o0/out DMAs (address-based), so no tile_position tricks.
  Phase 1 (node-sharded): g64 = 64*g (bf16 -> host casts fp8) and
  o0 = (1+eps)*feat.  Host gathers g64[src_e] and scatters RAW ew_e
  into the one-hot slots with -80 fill (pure relayout); device does
  stw = exp(beta*straw - ln64) = w_e/64 one-hot-placed, then
  h = stw^T @ ge64 as PSUM-accumulating matmuls.
"""

import sys

sys.path.insert(0, "/opt/trn_rl_repo")

import numpy as np

N, E, D = 50000, 640000, 128
NCORES = 8
SH = N // NCORES            # 6250 dst nodes per core
HTILES = (SH + 127) // 128  # 49 dst tiles per core
SHP = HTILES * 128          # 6272 padded nodes per core

PAD_EW = -80.0              # exp(beta*PAD_EW) == 0 (inside ACT LUT range)
LN64 = 4.1588830833596715
GRP = 4                     # max dst tiles per DMA/ACT batch
# group plan: small first group (fast pipeline fill) and small last
# group (short tail); 4-tile groups in between
GROUPS = [[0], [1, 2, 3]] + [
    list(range(4 + 4 * k, 8 + 4 * k)) for k in range(11)
] + [[48]]
NG = len(GROUPS)            # 14
GID = {}
for _gi, _g in enumerate(GROUPS):
    for _i in _g:
        GID[_i] = _gi


def _host_prep(src, dst, edge_weight):
    """Index/layout prep only (no float math on tensor values)."""
    import ml_dtypes

    src = np.asarray(src).astype(np.int64)
    dst = np.asarray(dst).astype(np.int64)
    ew = np.asarray(edge_weight).astype(np.float32)

    # ---- per-node src-grouped edge-weight rows (for z), fixed K ----
    deg = np.bincount(src, minlength=N)
    K = int(deg.max())
    order = np.argsort(src, kind="stable")
    starts = np.zeros(N + 1, np.int64)
    np.cumsum(deg, out=starts[1:])
    slot = np.arange(E, dtype=np.int64) - starts[src[order]]
    zpad = np.full((N, K), PAD_EW, np.float32)
    zpad[src[order], slot] = ew[order]

    # per-core phase-1 z rows: [128, HTILES*K] bf16
    zrows = []
    for c in range(NCORES):
        zm = np.full((SHP, K), PAD_EW, np.float32)
        zm[:SH] = zpad[c * SH : (c + 1) * SH]
        zrows.append(
            np.ascontiguousarray(
                zm.reshape(HTILES, 128, K).transpose(1, 0, 2)
            ).reshape(128, HTILES * K).astype(ml_dtypes.bfloat16)
        )

    # ---- per-core edge bucketing by (dst tile, dst half) ----
    owner = dst // SH
    dstl = dst - owner * SH
    dtile = dstl // 128
    dbit = dstl % 128
    half = dbit // 64
    dcol = dbit % 64
    bucket = dtile * 2 + half          # [0, 2*HTILES)

    counts = np.zeros((NCORES, 2 * HTILES), np.int64)
    np.add.at(counts, (owner, bucket), 1)
    net = (counts.max(axis=0) + 127) // 128  # edge tiles per bucket
    net = np.maximum(net, 1)

    # group-local layout: buckets of group g at local cumsum offsets,
    # each group padded to GMAX edge tiles
    loff = np.zeros(2 * HTILES, np.int64)
    gsum = np.zeros(NG, np.int64)
    for g, tiles in enumerate(GROUPS):
        o = 0
        for i in tiles:
            for h in (0, 1):
                loff[2 * i + h] = o
                o += int(net[2 * i + h])
        gsum[g] = o
    GMAX = int(gsum.max())
    EPAD = NG * GMAX * 128

    core_idx = []
    for c in range(NCORES):
        m = np.nonzero(owner == c)[0]
        key = bucket[m]
        korder = np.argsort(key, kind="stable")
        me = m[korder]
        keys = key[korder]
        kb = np.r_[0, np.nonzero(np.diff(keys))[0] + 1]
        sf = np.zeros(len(keys), np.int64)
        sf[kb] = kb
        np.maximum.accumulate(sf, out=sf)
        within = np.arange(len(keys)) - sf
        gid = np.array([GID[i] for i in range(HTILES)], np.int64)
        grp = gid[keys // 2]
        pos = (grp * GMAX + loff[keys]) * 128 + within

        src_pad = np.full(EPAD, N, np.int64)      # pad edges read zero row N

        # raw-ew scatter matrix: ew_e at [e, dcol_e], -80 fill (fp8).
        # layout [NG*128, GMAX*64]: group blocks fully contiguous.
        stf = np.full((EPAD, 64), PAD_EW, np.float32)
        stf[pos, dcol[me]] = ew[me]
        straw = np.ascontiguousarray(
            stf.reshape(NG, GMAX, 128, 64).transpose(0, 2, 1, 3)
        ).reshape(NG * 128, GMAX * 64).astype(ml_dtypes.float8_e4m3)

        src_pad[pos] = src[me]
        core_idx.append((src_pad, straw))

    return zrows, core_idx, net, K, loff, gsum, GMAX


_COMPILED = {}


def _build_phase1(K):
    import concourse.bass as bass
    import concourse.bacc as bacc
    from concourse import mybir, tile

    f32 = mybir.dt.float32
    bf16 = mybir.dt.bfloat16
    i32 = mybir.dt.int32
    AF = mybir.ActivationFunctionType
    ALU = mybir.AluOpType
    X = mybir.AxisListType.X

    NT = HTILES       # 49 node tiles
    CH = 25           # node tiles per pipeline chunk (2 chunks: 25+24)
    NCH = (NT + CH - 1) // CH

    nc = bacc.Bacc(None, debug=False)
    fm_ext = nc.dram_tensor("feat_my", [128, NT * D], bf16, kind="ExternalInput")
    zr_ext = nc.dram_tensor("zrow", [128, NT * K], bf16, kind="ExternalInput")
    beta_ext = nc.dram_tensor("beta128", [128, 1], f32, kind="ExternalInput")
    eps_ext = nc.dram_tensor("eps128", [128, 1], f32, kind="ExternalInput")
    g_ext = nc.dram_tensor("g64", [128, NT * D], bf16, kind="ExternalOutput")
    o0_ext = nc.dram_tensor("o0", [128, NT * D], bf16, kind="ExternalOutput")

    with tile.TileContext(nc) as tc:
        with (
            tc.tile_pool(name="pp", bufs=1) as pp,
            tc.tile_pool(name="fmp", bufs=2) as fmp,
            tc.tile_pool(name="zrp", bufs=2) as zrp,
            tc.tile_pool(name="sqp", bufs=2) as sqp,
            tc.tile_pool(name="smp", bufs=2 * 8) as smp,
            tc.tile_pool(name="gp", bufs=2) as gp,
            tc.tile_pool(name="op", bufs=2) as op,
        ):
            beta_b = pp.tile([128, 1], f32, tag="beta_b")
            nc.sync.dma_start(out=beta_b[:], in_=beta_ext[:])
            ep1_b = pp.tile([128, 1], f32, tag="ep1_b")
            nc.sync.dma_start(out=ep1_b[:], in_=eps_ext[:])
            nc.vector.tensor_scalar_add(ep1_b[:], ep1_b[:], 1.0)

            for ci in range(NCH):
                t0 = ci * CH
                nt = min(CH, NT - t0)
                fm = fmp.tile([128, CH, D], bf16, tag="fm")
                nc.sync.dma_start(
                    out=fm[:, :nt, :].rearrange("p a b -> p (a b)"),
                    in_=fm_ext[:, t0 * D : (t0 + nt) * D],
                )
                # zrow on the scalar HWDGE ring: feeds the first ACT
                # without queueing behind fm
                zr = zrp.tile([128, CH, K], bf16, tag="zr")
                nc.scalar.dma_start(
                    out=zr[:, :nt, :].rearrange("p a b -> p (a b)"),
                    in_=zr_ext[:, t0 * K : (t0 + nt) * K],
                )

                # o0 = (1+eps)*feat (ACT, independent of the norm chain --
                # emitted first so it overlaps and doesn't extend the tail)
                o0 = op.tile([128, CH, D], bf16, tag="o0")
                nc.scalar.activation(
                    o0[:, :nt, :].rearrange("p a b -> p (a b)"),
                    fm[:, :nt, :].rearrange("p a b -> p (a b)"),
                    AF.Copy,
                    scale=ep1_b[:],
                )
                nc.sync.dma_start(
                    out=o0_ext[:, t0 * D : (t0 + nt) * D],
                    in_=o0[:, :nt, :].rearrange("p a b -> p (a b)"),
                )

                # z = sum_k exp(beta * zrow_k)
                zx = zrp.tile([128, CH, K], bf16, tag="zx")
                nc.scalar.activation(
                    zx[:, :nt, :].rearrange("p a b -> p (a b)"),
                    zr[:, :nt, :].rearrange("p a b -> p (a b)"),
                    AF.Exp,
                    scale=beta_b[:],
                )
                z = smp.tile([128, CH], f32, tag="z")
                nc.vector.tensor_reduce(z[:, :nt], zx[:, :nt, :], X, ALU.add)

                # ss = ||feat||^2 per node (bf16 squares -> 2x mode)
                sq = sqp.tile([128, CH, D], bf16, tag="sq")
                nc.vector.tensor_tensor(
                    sq[:, :nt, :].rearrange("p a b -> p (a b)"),
                    fm[:, :nt, :].rearrange("p a b -> p (a b)"),
                    fm[:, :nt, :].rearrange("p a b -> p (a b)"),
                    ALU.mult,
                )
                ss = smp.tile([128, CH], f32, tag="ss")
                nc.vector.tensor_reduce(ss[:, :nt], sq[:, :nt, :], X, ALU.add)

                # rz = 64/sqrt(ss*z^2): one bit-hack rsqrt + Newton on v
                v = smp.tile([128, CH], f32, tag="v")
                nc.vector.tensor_tensor(v[:, :nt], ss[:, :nt], z[:, :nt], ALU.mult)
                nc.vector.tensor_tensor(v[:, :nt], v[:, :nt], z[:, :nt], ALU.mult)
                y0 = smp.tile([128, CH], f32, tag="y0")
                nc.vector.tensor_scalar(
                    y0[:, :nt].bitcast(i32), v[:, :nt].bitcast(i32),
                    1, -1, op0=ALU.arith_shift_right, op1=ALU.bitwise_xor,
                )
                nc.vector.tensor_scalar(
                    y0[:, :nt].bitcast(i32), y0[:, :nt].bitcast(i32),
                    0x5F3759E0, None, op0=ALU.add,
                )
                u = smp.tile([128, CH], f32, tag="u")
                nc.vector.tensor_tensor(u[:, :nt], y0[:, :nt], y0[:, :nt], ALU.mult)
                nc.vector.tensor_tensor(u[:, :nt], u[:, :nt], v[:, :nt], ALU.mult)
                # fold the 64x: y = y*(96 - 32*u) = 64*y*(1.5 - 0.5*u)
                nc.vector.tensor_scalar(
                    u[:, :nt], u[:, :nt], -32.0, 96.0, op0=ALU.mult, op1=ALU.add
                )
                rz = smp.tile([128, CH], f32, tag="rz")
                nc.vector.tensor_tensor(rz[:, :nt], y0[:, :nt], u[:, :nt], ALU.mult)

                # g64 = feat * rz
                g = gp.tile([128, CH, D], bf16, tag="g")
                rzb = rz[:, :nt].unsqueeze(2).broadcast_to([128, nt, D])
                nc.vector.tensor_tensor(g[:, :nt, :], fm[:, :nt, :], rzb, ALU.mult)
                nc.sync.dma_start(
                    out=g_ext[:, t0 * D : (t0 + nt) * D],
                    in_=g[:, :nt, :].rearrange("p a b -> p (a b)"),
                )

    nc.finalize()
    return nc


def _build_phase2(net, loff, gsum, GMAX):
    import concourse.bass as bass
    import concourse.bacc as bacc
    from concourse import mybir, tile

    f32 = mybir.dt.float32
    bf16 = mybir.dt.bfloat16
    f8 = mybir.dt.float8e4
    AF = mybir.ActivationFunctionType
    ALU = mybir.AluOpType

    NT = HTILES

    nc = bacc.Bacc(None, debug=False)
    ge_ext = nc.dram_tensor("ge", [NG * 128, GMAX * D], f8, kind="ExternalInput")
    st_ext = nc.dram_tensor("straw", [NG * 128, GMAX * 64], f8, kind="ExternalInput")
    o0_ext = nc.dram_tensor("o0", [128, NT * D], bf16, kind="ExternalInput")
    beta_ext = nc.dram_tensor("beta128", [128, 1], f32, kind="ExternalInput")
    # out layout [64, (group, half, grp-local, D)]: one DMA per group
    out_ext = nc.dram_tensor(
        "out", [64, NG * 2 * GRP * D], bf16, kind="ExternalOutput"
    )

    with tile.TileContext(nc) as tc:
        with (
            tc.tile_pool(name="persist", bufs=1) as pp,
            tc.tile_pool(name="gep", bufs=4) as gepool,
            tc.tile_pool(name="stp", bufs=4) as stpool,
            tc.tile_pool(name="stw", bufs=5) as stwpool,
            tc.tile_pool(name="outp", bufs=3) as opool,
            tc.tile_pool(name="hpsum", bufs=8, space="PSUM") as hpsum,
        ):
            beta_b = pp.tile([128, 1], f32, tag="beta_b")
            nc.sync.dma_start(out=beta_b[:], in_=beta_ext[:])
            bl64 = pp.tile([128, 1], f32, tag="bl64")
            nc.vector.memset(bl64[:], -LN64)

            # (1+eps)*feat resident in thirds, loads staggered across early
            # groups so the o0 traffic never saturates the SDMA engines at
            # startup (a consolidated 2.4MB burst stalled group 1 by 14us).
            # A-half reads rows 0:64 of the full-partition part tiles; the
            # B-half needs rows 64:128 shifted to partitions 0:64, done by
            # the HBM load itself (DMA is address-based).
            O0SPLIT = [(0, 49, 0, 0)]
            o0a_parts = []
            o0b_parts = []
            for ps, pe, _, _ in O0SPLIT:
                o0a_p = pp.tile([128, pe - ps, D], bf16, tag=f"o0a{ps}")
                o0b_p = pp.tile([64, pe - ps, D], bf16, tag=f"o0b{ps}")
                o0a_parts.append(o0a_p)
                o0b_parts.append(o0b_p)

            def o0_lookup(i):
                for pi, (ps, pe, _, _) in enumerate(O0SPLIT):
                    if i < pe:
                        return pi, i - ps
                raise AssertionError

            for g, tiles in enumerate(GROUPS):
                i0 = tiles[0]
                nw = len(tiles)
                gnh = int(gsum[g])

                # straw rides the scalar-issued HWDGE ring so it never
                # queues behind the bigger ge stream -> ACT stays ahead
                straw = stpool.tile([128, GMAX, 64], f8, tag="straw")
                nc.scalar.dma_start(
                    out=straw[:, :gnh, :].rearrange("p a b -> p (a b)"),
                    in_=st_ext[g * 128 : (g + 1) * 128, : gnh * 64],
                )
                ge = gepool.tile([128, GMAX, D], f8, tag="ge")
                nc.sync.dma_start(
                    out=ge[:, :gnh, :].rearrange("p a b -> p (a b)"),
                    in_=ge_ext[g * 128 : (g + 1) * 128, : gnh * D],
                )
                for pi, (ps, pe, ga, gb) in enumerate(O0SPLIT):
                    if g == ga:
                        nc.scalar.dma_start(
                            out=o0a_parts[pi][:].rearrange("p a b -> p (a b)"),
                            in_=o0_ext[:, ps * D : pe * D],
                        )
                    if g == gb:
                        nc.sync.dma_start(
                            out=o0b_parts[pi][:].rearrange("p a b -> p (a b)"),
                            in_=o0_ext[64:128, ps * D : pe * D],
                        )

                # stw = exp(beta*straw - ln64): w_e/64 one-hot-placed
                stw = stwpool.tile([128, GMAX, 64], bf16, tag="stw")
                nc.scalar.activation(
                    stw[:, :gnh, :].rearrange("p a b -> p (a b)"),
                    straw[:, :gnh, :].rearrange("p a b -> p (a b)"),
                    AF.Exp,
                    bias=bl64[:],
                    scale=beta_b[:],
                )

                og = opool.tile([64, 2, GRP, D], bf16, tag="og")
                for li, i in enumerate(tiles):
                    for h in (0, 1):
                        nh = int(net[2 * i + h])
                        lo = int(loff[2 * i + h])
                        hp = hpsum.tile([64, D], f32, tag="hp")
                        for t in range(nh):
                            nc.tensor.matmul(
                                hp[:],
                                stw[:, lo + t, :],
                                ge[:, lo + t, :],
                                start=(t == 0),
                                stop=(t == nh - 1),
                            )
                        pi, io = o0_lookup(i)
                        o0src = (
                            o0a_parts[pi][0:64, io, :]
                            if h == 0
                            else o0b_parts[pi][:, io, :]
                        )
                        nc.vector.tensor_tensor(
                            og[:, h, li, :], o0src, hp[:], ALU.add
                        )
                nc.sync.dma_start(
                    out=out_ext[:, g * 2 * GRP * D : (g + 1) * 2 * GRP * D],
                    in_=og[:].rearrange("p a b c -> p (a b c)"),
                )

    nc.finalize()
    return nc


def kernel(feat, edge_weight, src, dst, beta, eps):
    from concourse.bass_utils import run_bass_kernel_spmd
    import ml_dtypes

    feat = np.asarray(feat, dtype=np.float32)
    ew = np.asarray(edge_weight, dtype=np.float32)
    beta = np.asarray(beta, dtype=np.float32)
    eps = np.asarray(eps, dtype=np.float32)

    zrows, core_idx, net, K, loff, gsum, GMAX = _host_prep(src, dst, ew)

    key = (K, GMAX, tuple(int(x) for x in net))
    if key not in _COMPILED:
        _COMPILED[key] = (
            _build_phase1(K),
            _build_phase2(net, loff, gsum, GMAX),
        )
    nc1, nc2 = _COMPILED[key]

    beta128 = np.ascontiguousarray(np.broadcast_to(beta.reshape(1, 1), (128, 1)))
    eps128 = np.ascontiguousarray(np.broadcast_to(eps.reshape(1, 1), (128, 1)))

    # ---------------- phase 1: per-node g64, o0 ----------------
    in1 = []
    for c in range(NCORES):
        fmp = np.zeros((SHP, D), np.float32)
        fmp[:SH] = feat[c * SH : (c + 1) * SH]
        fmt = np.ascontiguousarray(
            fmp.reshape(HTILES, 128, D).transpose(1, 0, 2)
        ).reshape(128, HTILES * D).astype(ml_dtypes.bfloat16)
        in1.append(
            {"feat_my": fmt, "zrow": zrows[c], "beta128": beta128,
             "eps128": eps128}
        )

    res1 = run_bass_kernel_spmd(nc1, in1, core_ids=list(range(NCORES)))
    gfull = np.empty((N + 1, D), dtype=ml_dtypes.float8_e4m3)
    o0s = []
    for c in range(NCORES):
        gc = np.asarray(res1.results[c]["g64"]).reshape(128, HTILES, D)
        gfull[c * SH : (c + 1) * SH] = (
            gc.transpose(1, 0, 2).reshape(SHP, D)[:SH].astype(ml_dtypes.float8_e4m3)
        )
        o0s.append(np.asarray(res1.results[c]["o0"]))
    gfull[N] = 0  # pad row

    # ---------------- host gather of g64[src_e] ----------------
    in2 = []
    for c in range(NCORES):
        src_pad, straw = core_idx[c]
        ge = np.ascontiguousarray(
            gfull[src_pad].reshape(NG, GMAX, 128, D).transpose(0, 2, 1, 3)
        ).reshape(NG * 128, GMAX * D)
        in2.append(
            {"ge": ge, "straw": straw, "o0": o0s[c], "beta128": beta128}
        )

    res2 = run_bass_kernel_spmd(nc2, in2, core_ids=list(range(NCORES)))
    out = np.empty((N, D), np.float32)
    full = np.empty((SHP, D), np.float32)
    for c in range(NCORES):
        # [64, NG, 2, GRP, D] -> node i*128 + h*64 + p
        oc = np.asarray(res2.results[c]["out"]).reshape(64, NG, 2, GRP, D)
        for gi, tiles in enumerate(GROUPS):
            for li, i in enumerate(tiles):
                for h in (0, 1):
                    full[i * 128 + h * 64 : i * 128 + (h + 1) * 64] = oc[
                        :, gi, h, li, :
                    ]
        out[c * SH : (c + 1) * SH] = full[:SH]
    return out
